# revision 19
# baseline (speedup 1.0000x reference)
"""TARDIS decoder Bass kernel for Trainium2, 8-way batch-parallel.

Strategy (per core, 16 batch elements):
- Keep M = mem @ W_m2w resident in SBUF as [k(2x128 part), (b=16, n=512) free];
  memory writes touch one slot per batch element per step, so M is updated
  incrementally (delta @ W_m2w[A:]) instead of recomputing the batched matmul.
- Hard gumbel-softmax == argmax(logits + g); tau/softplus drop out of the
  forward pass entirely.  Gumbel noise is reproduced host-side (fixed key 42).
- logits = sum_k base_k * tanh(c[b,k] + M[b,n,k]): tanh on ScalarE with the
  per-(b, k-tile) bias trick; the k-reduction runs on PE with masked-lhsT
  accumulation into one [16,512] PSUM tile.
- LayerNorm of w_sum reduces to an integer-indexed rsqrt LUT gather because
  w_sum is a sum of exact one-hots (mean = t/512 is a compile-time constant).
- Memory content lives in DRAM (C_mem) accessed via indirect row gather/
  scatter on the gpsimd dynamic queue; the address-bits contribution of every
  read comes from precomputed tables (addr @ W_r2*) gathered the same way.
"""

import contextlib
import ctypes
import os
import sys
import types

sys.path.insert(0, "/opt/trn_rl_repo")

import numpy as np

import bass_rust
import concourse.bass as bass
import concourse.tile as tile
from concourse import mybir
from concourse.bass_utils import run_bass_kernel_spmd
from concourse.masks import make_identity

dt = mybir.dt

L, B, IDIM, HDIM, N, A, C = 32, 128, 512, 1024, 512, 128, 128
AC = A + C
NCORES = 8
BL = B // NCORES  # 16
JH = HDIM // 128  # 8
KT = AC // 128    # 2
JI = IDIM // 128  # 4
NSTEPS = int(os.environ.get("TARDIS_STEPS", str(L)))


def _install_ntff_hook():
    """Register the axon NTFF profiling hook (missing antenv.axon_hooks shim)."""
    if "antenv.axon_hooks" in sys.modules:
        return
    so_path = "/opt/axon/libaxon_pjrt.so"
    try:
        lib = ctypes.CDLL(so_path)
        lib.axon_start_nrt_profile.argtypes = [
            ctypes.POINTER(ctypes.c_int64), ctypes.c_size_t]
        lib.axon_start_nrt_profile.restype = ctypes.c_int64
        lib.axon_stop_nrt_profile.argtypes = [ctypes.c_char_p]
        lib.axon_stop_nrt_profile.restype = ctypes.c_int64
    except OSError:
        return

    @contextlib.contextmanager
    def _hook(output_dir, device_ids):
        import jax
        jax.devices()
        if device_ids:
            ids = (ctypes.c_int64 * len(device_ids))(*device_ids)
            rc = lib.axon_start_nrt_profile(ids, len(device_ids))
        else:
            rc = lib.axon_start_nrt_profile(None, 0)
        if rc != 0:
            raise RuntimeError(f"axon_start_nrt_profile rc={rc}")
        try:
            yield
        finally:
            n = lib.axon_stop_nrt_profile(str(output_dir).encode())
            if n < 0:
                raise RuntimeError(f"axon_stop_nrt_profile rc={n}")

    mod = types.ModuleType("antenv.axon_hooks")
    mod.get_axon_ntff_profile_hook = lambda: _hook
    mod.set_axon_ntff_profile_hook = lambda h: None
    sys.modules["antenv.axon_hooks"] = mod


_install_ntff_hook()


def split_multi_waits(nc):
    """This container's walrus accepts only one sync-wait per instruction;
    hoist extra waits onto preceding NOPs on the same engine."""
    nsplit = 0
    for f in nc.m.functions:
        for blk in f.blocks:
            insts = blk.instructions
            newlist = []
            for inst in insts:
                si = inst.sync_info
                if si is not None and si.on_wait and len(si.on_wait) > 1:
                    waits = list(si.on_wait)
                    for w in waits[:-1]:
                        nop = mybir.InstNoOp(name=f"waitsplit_{nc.next_id()}")
                        nop.engine = inst.engine
                        nop.sync_info = bass_rust.SyncInfo(on_wait=[w], on_update=[])
                        newlist.append(nop)
                        nsplit += 1
                    si.on_wait = [waits[-1]]
                    inst.sync_info = si
                newlist.append(inst)
            insts[:] = newlist
    return nsplit


# ---------------------------------------------------------------- device code


def build_program():
    nc = bass.Bass(trn_type="TRN2")
    f32, i32, u32 = dt.float32, dt.int32, dt.uint32
    TANH = mybir.ActivationFunctionType.Tanh
    SIG = mybir.ActivationFunctionType.Sigmoid
    ADD = mybir.AluOpType.add
    SUB = mybir.AluOpType.subtract
    MULT = mybir.AluOpType.mult
    ISEQ = mybir.AluOpType.is_equal
    ISGT = mybir.AluOpType.is_gt

    # -------- DRAM I/O (names = in_map keys)
    d_inpT = nc.dram_tensor("inpT", [128, JI * L * BL], f32, kind="ExternalInput")
    d_h0T = nc.dram_tensor("h0T", [128, JH * BL], f32, kind="ExternalInput")
    d_Wicat = nc.dram_tensor("Wicat", [128, JI * 1285], f32, kind="ExternalInput")
    d_Whw = nc.dram_tensor("Whw", [128, JH * AC], f32, kind="ExternalInput")
    d_Whc = nc.dram_tensor("Whc", [128, JH * HDIM], f32, kind="ExternalInput")
    d_Whgab = nc.dram_tensor("Whgab", [128, JH * 5], f32, kind="ExternalInput")
    d_Whm = nc.dram_tensor("Whm", [128, JH * C], f32, kind="ExternalInput")
    d_Wrc = nc.dram_tensor("Wrc", [128, 1029], f32, kind="ExternalInput")
    d_Wm2A = nc.dram_tensor("Wm2A", [128, AC], f32, kind="ExternalInput")
    d_M0T = nc.dram_tensor("M0T", [128, KT * N], f32, kind="ExternalInput")
    d_bmask = nc.dram_tensor("bmask", [128, KT * BL * BL], f32, kind="ExternalInput")
    d_colsum = nc.dram_tensor("colsum16", [BL, AC], f32, kind="ExternalInput")
    d_iota = nc.dram_tensor("iota16", [BL, N], f32, kind="ExternalInput")
    d_boff = nc.dram_tensor("boff", [BL, 1], i32, kind="ExternalInput")
    d_g = nc.dram_tensor("g_read", [L * BL, N], f32, kind="ExternalInput")
    d_gdT = nc.dram_tensor("gdT", [2, L * BL], f32, kind="ExternalInput")
    d_Tcat = nc.dram_tensor("Tcat", [N, 1029], f32, kind="ExternalInput")
    d_Wu2w = nc.dram_tensor("Wu2w", [N, AC], f32, kind="ExternalInput")
    d_lut = nc.dram_tensor("slut", [(L + 1) * 1025, 1], f32, kind="ExternalInput")
    d_sel5 = nc.dram_tensor("sel5", [5, 5 * 128], f32, kind="ExternalInput")
    d_M0rows = nc.dram_tensor("M0rows", [N, AC], f32, kind="ExternalInput")
    d_base16 = nc.dram_tensor("base16", [128, KT * BL], f32, kind="ExternalInput")
    d_bsel = nc.dram_tensor("bsel128", [128, BL], f32, kind="ExternalInput")

    d_out = nc.dram_tensor("outs", [L, HDIM, BL], f32, kind="ExternalOutput")

    d_cmem = nc.dram_tensor("C_mem", [BL * N, C], f32, kind="Internal")

    with tile.TileContext(nc) as tc, contextlib.ExitStack() as ctx:
        cpool = ctx.enter_context(tc.tile_pool(name="const", bufs=1))
        spool = ctx.enter_context(tc.tile_pool(name="state", bufs=1))

        # ---------------- persistent constants
        Whw = cpool.tile([128, JH * AC], f32)
        Whc = cpool.tile([128, JH * HDIM], f32)
        Whgab = cpool.tile([128, JH * 5], f32)
        Whm = cpool.tile([128, JH * C], f32)
        Wrc = cpool.tile([128, 1029], f32)
        Wm2A = cpool.tile([128, AC], f32)
        bmask = cpool.tile([128, KT * BL * BL], f32)
        colsum16 = cpool.tile([BL, AC], f32)
        iota16 = cpool.tile([BL, N], f32)
        boff = cpool.tile([BL, 1], i32)
        gdT = cpool.tile([2, L * BL], f32)
        sel5 = cpool.tile([5, 5 * 128], f32)
        base16 = cpool.tile([128, KT * BL], f32)
        bsel128 = cpool.tile([128, BL], f32)
        ident16 = cpool.tile([16, 16], f32)
        ident128 = cpool.tile([128, 128], f32)
        E_wT = cpool.tile([128, KT * L * BL], f32)     # 4 KB/part
        E_cT = cpool.tile([128, JH * L * BL], f32)     # 16 KB/part
        E_gT = cpool.tile([3, L * BL], f32)
        E_abT = cpool.tile([2, L * BL], f32)

        for t_, d_ in [(Whw, d_Whw), (Whc, d_Whc), (Whgab, d_Whgab),
                       (Whm, d_Whm), (Wrc, d_Wrc), (Wm2A, d_Wm2A),
                       (bmask, d_bmask), (colsum16, d_colsum),
                       (iota16, d_iota), (boff, d_boff), (gdT, d_gdT),
                       (sel5, d_sel5), (base16, d_base16), (bsel128, d_bsel)]:
            nc.sync.dma_start(t_[:], d_[:])
        make_identity(nc, ident16[:])
        make_identity(nc, ident128[:])

        # ---------------- persistent state
        hT = spool.tile([128, JH * BL], f32)       # h^T  [128,(8,16)]
        ccT = spool.tile([128, JH * BL], f32)      # cc^T
        M = [spool.tile([128, BL * N], f32, name=f"M{_k}") for _k in range(KT)]  # 32 KB/part each
        U = spool.tile([BL, AC], f32)
        w_sum = spool.tile([BL, N], f32)
        sq = spool.tile([BL, 1], f32)
        s_cur = spool.tile([BL, 1], f32)
        # telescoping-correction pair state: columns j = (t_w-16)*16 + b
        NPAIR = (L - 16) * BL  # 256
        SoldT = [spool.tile([128, NPAIR], f32, name=f"SoldT{_k}") for _k in range(KT)]
        SnewT = [spool.tile([128, NPAIR], f32, name=f"SnewT{_k}") for _k in range(KT)]
        oneh = [spool.tile([128, N], f32, name=f"oneh{_k}") for _k in range(NPAIR // 128)]

        nc.sync.dma_start(hT[:], d_h0T[:])
        nc.vector.memset(ccT[:], 0.0)
        nc.vector.memset(U[:], 0.0)
        nc.vector.memset(w_sum[:], 0.0)
        nc.vector.memset(sq[:], 0.0)
        nc.vector.memset(s_cur[:], float(np.float32(1.0) / np.sqrt(np.float32(1e-5))))

        # M init: M[kt][:, b*N:(b+1)*N] = M0T[:, kt*N:(kt+1)*N] for each b
        for kt in range(KT):
            for b in range(BL):
                nc.sync.dma_start(M[kt][:, b * N:(b + 1) * N],
                                  d_M0T[:, kt * N:(kt + 1) * N])

        # C_mem zero-fill (16 x 512 rows), keep insts for an explicit dep
        zfill = []
        with tc.tile_pool(name="zt", bufs=1) as zpool:
            zt = zpool.tile([128, 512], f32)
            nc.vector.memset(zt[:], 0.0)
            for a in range(BL):
                cm_v = d_cmem[a * 512:(a + 1) * 512, :].rearrange(
                    "(p q) c -> p (q c)", p=128)
                zfill.append(nc.sync.dma_start(cm_v, zt[:]))

        # ---------------- E-precompute: E_*T = (inp @ W_i*)^T, feature-major
        with tc.tile_pool(name="pre", bufs=1) as prepool, \
             tc.tile_pool(name="prepsum", bufs=4, space="PSUM") as prepsum:
            inpT = prepool.tile([128, JI * L * BL], f32)
            Wicat = prepool.tile([128, JI * 1285], f32)
            nc.sync.dma_start(inpT[:], d_inpT[:])
            nc.sync.dma_start(Wicat[:], d_Wicat[:])
            inpT_v = inpT[:].rearrange("p (j lb) -> p j lb", j=JI)
            Wic_v = Wicat[:].rearrange("p (j m) -> p j m", j=JI)
            # feature tiles: 0..1 -> E_wT, 2..9 -> E_cT, then gates [3], ab [2]
            for mt in range(12):
                if mt < 10:
                    m0, mw = mt * 128, 128
                elif mt == 10:
                    m0, mw = 1280, 3
                else:
                    m0, mw = 1283, 2
                pe = prepsum.tile([128, L * BL], f32, tag="pe")
                for j in range(JI):
                    nc.tensor.matmul(
                        out=pe[:mw, :], lhsT=Wic_v[:, j, m0:m0 + mw],
                        rhs=inpT_v[:, j, :], start=(j == 0), stop=(j == JI - 1))
                if mt < 2:
                    nc.vector.tensor_copy(
                        E_wT[:].rearrange("p (k lb) -> p k lb", k=KT)[:, mt, :],
                        pe[:, :])
                elif mt < 10:
                    nc.vector.tensor_copy(
                        E_cT[:].rearrange("p (j lb) -> p j lb", j=JH)[:, mt - 2, :],
                        pe[:, :])
                elif mt == 10:
                    nc.vector.tensor_copy(E_gT[:], pe[:3, :])
                else:
                    nc.vector.tensor_copy(E_abT[:], pe[:2, :])

        # ---------------- main pools
        wpool = ctx.enter_context(tc.tile_pool(name="work", bufs=1))
        tpool = ctx.enter_context(tc.tile_pool(name="tanh", bufs=4))
        gpool = ctx.enter_context(tc.tile_pool(name="gbuf", bufs=2))
        plog_p = ctx.enter_context(tc.tile_pool(name="plog", bufs=2, space="PSUM"))
        ps_p = ctx.enter_context(tc.tile_pool(name="ps", bufs=6, space="PSUM"))

        hT_v = hT[:].rearrange("p (j b) -> p j b", j=JH)
        ccT_v = ccT[:].rearrange("p (j b) -> p j b", j=JH)
        Whw_v = Whw[:].rearrange("p (j m) -> p j m", j=JH)
        Whc_v = Whc[:].rearrange("p (j m) -> p j m", j=JH)
        Whgab_v = Whgab[:].rearrange("p (j m) -> p j m", j=JH)
        Whm_v = Whm[:].rearrange("p (j m) -> p j m", j=JH)
        E_wT_v = E_wT[:].rearrange("p (k l b) -> p k l b", k=KT, l=L)
        E_cT_v = E_cT[:].rearrange("p (j l b) -> p j l b", j=JH, l=L)
        bmask_v = bmask[:].rearrange("p (k b m) -> p k b m", k=KT, b=BL)

        first_gather_insts = []

        for t in range(NSTEPS):
            mu_t = np.float32(t) / np.float32(N)

            # ---- cT = (h @ W_h2w)^T + E_wT[t] + diag(s)-terms   [k, b]
            Um = wpool.tile([BL, AC], f32, tag="Um")
            # U_minus = U - mu_t * colsum  (mu_0 = 0 but keep uniform)
            nc.vector.tensor_scalar(
                out=Um[:], in0=colsum16[:], scalar1=-float(mu_t), scalar2=None,
                op0=MULT)
            nc.vector.tensor_tensor(out=Um[:], in0=Um[:], in1=U[:], op=ADD)
            diag_s = wpool.tile([BL, BL], f32, tag="diag")
            nc.vector.tensor_scalar(
                out=diag_s[:], in0=ident16[:], scalar1=s_cur[:, 0:1], scalar2=None,
                op0=MULT)
            cT = [wpool.tile([128, BL], f32, name=f"cT{_k}", tag=f"cT{_k}") for _k in range(KT)]
            for kt in range(KT):
                pc = ps_p.tile([128, BL], f32, tag="ps")
                for j in range(JH):
                    nc.tensor.matmul(
                        out=pc[:], lhsT=Whw_v[:, j, kt * 128:(kt + 1) * 128],
                        rhs=hT_v[:, j, :], start=(j == 0), stop=False)
                nc.tensor.matmul(
                    out=pc[:], lhsT=Um[:, kt * 128:(kt + 1) * 128], rhs=diag_s[:],
                    start=False, stop=True)
                nc.vector.tensor_tensor(
                    out=cT[kt][:], in0=pc[:], in1=E_wT_v[:, kt, t, :], op=ADD)

            # ---- big tanh + masked-lhsT logits accumulation
            J = max(0, (t - 16)) * BL  # live correction pairs
            n_corr_mm = (J + 127) // 128
            plog = plog_p.tile([BL, N], f32, tag="plog")
            n_mm = 0
            for b in range(BL):
                for kt in range(KT):
                    th = tpool.tile([128, N], f32, tag="th")
                    nc.scalar.activation(
                        th[:], M[kt][:, b * N:(b + 1) * N], TANH,
                        bias=cT[kt][:, b:b + 1], scale=1.0)
                    nc.tensor.matmul(
                        out=plog[:], lhsT=bmask_v[:, kt, b, :], rhs=th[:],
                        start=(n_mm == 0),
                        stop=(n_mm == 2 * BL - 1 and n_corr_mm == 0))
                    n_mm += 1

            # ---- telescoping corrections for post-step-16 writes
            if J > 0:
                G = J // BL
                dts = []
                for kt in range(KT):
                    cbc = cT[kt][:].rearrange(
                        "p (one b) -> p one b", one=1).to_broadcast([128, G, BL])
                    argn = wpool.tile([128, NPAIR], f32, tag=f"argn{kt}")
                    argo = wpool.tile([128, NPAIR], f32, tag=f"argo{kt}")
                    argn_v = argn[:, :J].rearrange("p (g b) -> p g b", g=G)
                    argo_v = argo[:, :J].rearrange("p (g b) -> p g b", g=G)
                    nc.vector.tensor_tensor(
                        out=argn_v, in0=SnewT[kt][:, :J].rearrange(
                            "p (g b) -> p g b", g=G), in1=cbc, op=ADD)
                    nc.vector.tensor_tensor(
                        out=argo_v, in0=SoldT[kt][:, :J].rearrange(
                            "p (g b) -> p g b", g=G), in1=cbc, op=ADD)
                    nc.scalar.activation(argn[:, :J], argn[:, :J], TANH)
                    nc.scalar.activation(argo[:, :J], argo[:, :J], TANH)
                    nc.vector.tensor_tensor(
                        out=argn[:, :J], in0=argn[:, :J], in1=argo[:, :J], op=SUB)
                    dts.append(argn)
                pout1 = ps_p.tile([BL, NPAIR], f32, tag="ps")
                for kt in range(KT):
                    nc.tensor.matmul(
                        out=pout1[:, :J], lhsT=base16[:, kt * BL:(kt + 1) * BL],
                        rhs=dts[kt][:, :J], start=(kt == 0), stop=(kt == KT - 1))
                out1_sb = wpool.tile([BL, NPAIR], f32, tag="out1sb")
                nc.vector.tensor_copy(out1_sb[:, :J], pout1[:, :J])
                for ch in range(n_corr_mm):
                    chJ = min(128, J - ch * 128)
                    ptc = ps_p.tile([128, BL], f32, tag="ps")
                    nc.tensor.transpose(
                        out=ptc[:chJ, :],
                        in_=out1_sb[:, ch * 128:ch * 128 + chJ], identity=ident16[:])
                    lhsT2 = wpool.tile([128, BL], f32, tag="lhsT2")
                    nc.vector.tensor_tensor(
                        out=lhsT2[:chJ, :], in0=ptc[:chJ, :], in1=bsel128[:chJ, :],
                        op=MULT)
                    nc.tensor.matmul(
                        out=plog[:], lhsT=lhsT2[:chJ, :], rhs=oneh[ch][:chJ, :],
                        start=False, stop=(ch == n_corr_mm - 1))

            # ---- h @ W_h2c during the tanh window (no dep on logits)
            phwc = ps_p.tile([128, JH * BL], f32, tag="ps")
            phwc_v = phwc[:].rearrange("p (j b) -> p j b", j=JH)
            for m in range(JH):
                for j in range(JH):
                    nc.tensor.matmul(
                        out=phwc_v[:, m, :], lhsT=Whc_v[:, j, m * 128:(m + 1) * 128],
                        rhs=hT_v[:, j, :], start=(j == 0), stop=(j == JH - 1))
            # h @ W_h2gates -> [3, b]; h @ W_h2ab -> [2, b]
            pgt = ps_p.tile([3, BL], f32, tag="ps")
            pab = ps_p.tile([2, BL], f32, tag="ps")
            for j in range(JH):
                nc.tensor.matmul(
                    out=pgt[:], lhsT=Whgab_v[:, j, 0:3], rhs=hT_v[:, j, :],
                    start=(j == 0), stop=False)
                nc.tensor.matmul(
                    out=pab[:], lhsT=Whgab_v[:, j, 3:5], rhs=hT_v[:, j, :],
                    start=(j == 0), stop=False)

            # ---- logits + g, argmax
            g_t = gpool.tile([BL, N], f32, tag="g")
            nc.sync.dma_start(g_t[:], d_g[t * BL:(t + 1) * BL, :])
            logits = wpool.tile([BL, N], f32, tag="logits")
            nc.vector.tensor_tensor(out=logits[:], in0=plog[:], in1=g_t[:], op=ADD)
            mx8 = wpool.tile([BL, 8], f32, tag="mx8")
            mi8 = wpool.tile([BL, 8], u32, tag="mi8")
            nc.vector.max(out=mx8[:], in_=logits[:])
            nc.vector.max_index(out=mi8[:], in_max=mx8[:], in_values=logits[:])
            pos_i = wpool.tile([BL, 1], i32, tag="posi")
            nc.vector.tensor_copy(pos_i[:], mi8[:, 0:1])
            pos_f = wpool.tile([BL, 1], f32, tag="posf")
            nc.vector.tensor_copy(pos_f[:], mi8[:, 0:1])

            # ---- one-hot w, v, state updates (read side)
            w = wpool.tile([BL, N], f32, tag="w")
            nc.vector.tensor_scalar(
                out=w[:], in0=iota16[:], scalar1=pos_f[:, 0:1], scalar2=None,
                op0=ISEQ)
            wwsum = wpool.tile([BL, N], f32, tag="wwsum")
            v = wpool.tile([BL, 1], f32, tag="v")
            nc.vector.tensor_tensor(out=wwsum[:], in0=w[:], in1=w_sum[:], op=MULT)
            nc.vector.tensor_reduce(
                out=v[:], in_=wwsum[:], axis=mybir.AxisListType.X, op=ADD)
            nc.vector.tensor_tensor(out=w_sum[:], in0=w_sum[:], in1=w[:], op=ADD)
            # sq += 2v + 1 ; lut index = sq + (t+1)*1025 (values are exact ints)
            v21 = wpool.tile([BL, 1], f32, tag="v21")
            nc.vector.tensor_scalar(
                out=v21[:], in0=v[:], scalar1=2.0, scalar2=1.0, op0=MULT, op1=ADD)
            nc.vector.tensor_tensor(out=sq[:], in0=sq[:], in1=v21[:], op=ADD)
            lut_if = wpool.tile([BL, 1], f32, tag="lutif")
            nc.vector.tensor_scalar(
                out=lut_if[:], in0=sq[:], scalar1=float((t + 1) * 1025) + 0.5,
                scalar2=None, op0=ADD)
            lut_ii = wpool.tile([BL, 1], i32, tag="lutii")
            nc.vector.tensor_copy(lut_ii[:], lut_if[:])
            gi = nc.gpsimd.indirect_dma_start(
                out=s_cur[:], out_offset=None, in_=d_lut[:],
                in_offset=bass.IndirectOffsetOnAxis(ap=lut_ii[:, :1], axis=0))

            # ---- gathers at pos
            cm_idx = wpool.tile([BL, 1], i32, tag="cmidx")
            nc.vector.tensor_tensor(out=cm_idx[:], in0=pos_i[:], in1=boff[:], op=ADD)
            r_c = wpool.tile([BL, C], f32, tag="rc")
            g1 = nc.gpsimd.indirect_dma_start(
                out=r_c[:], out_offset=None, in_=d_cmem[:],
                in_offset=bass.IndirectOffsetOnAxis(ap=cm_idx[:, :1], axis=0))
            if t == 0:
                first_gather_insts.append(g1)
            Tg = wpool.tile([BL, 1029], f32, tag="Tg")
            nc.gpsimd.indirect_dma_start(
                out=Tg[:], out_offset=None, in_=d_Tcat[:],
                in_offset=bass.IndirectOffsetOnAxis(ap=pos_i[:, :1], axis=0))
            wu = wpool.tile([BL, AC], f32, tag="wu")
            nc.gpsimd.indirect_dma_start(
                out=wu[:], out_offset=None, in_=d_Wu2w[:],
                in_offset=bass.IndirectOffsetOnAxis(ap=pos_i[:, :1], axis=0))
            nc.vector.tensor_tensor(out=U[:], in0=U[:], in1=wu[:], op=ADD)

            # ---- r_c^T via PE transpose
            ptr = ps_p.tile([128, BL], f32, tag="ps")
            nc.tensor.transpose(out=ptr[:], in_=r_c[:], identity=ident16[:])
            r_cT = wpool.tile([128, BL], f32, tag="rcT")
            nc.vector.tensor_copy(r_cT[:], ptr[:])

            # ---- gates / ab  (feature-major, base-0 tiles)
            nc.tensor.matmul(out=pgt[:], lhsT=Wrc[:, 1024:1027], rhs=r_cT[:],
                             start=False, stop=False)
            nc.tensor.matmul(out=pgt[:], lhsT=Tg[:, 1024:1027], rhs=ident16[:],
                             start=False, stop=True)
            nc.tensor.matmul(out=pab[:], lhsT=Wrc[:, 1027:1029], rhs=r_cT[:],
                             start=False, stop=False)
            nc.tensor.matmul(out=pab[:], lhsT=Tg[:, 1027:1029], rhs=ident16[:],
                             start=False, stop=True)
            gt3 = wpool.tile([3, BL], f32, tag="gt3")
            nc.vector.tensor_tensor(
                out=gt3[:], in0=pgt[:], in1=E_gT[:, t * BL:(t + 1) * BL], op=ADD)
            nc.scalar.activation(gt3[:], gt3[:], SIG)
            ab2 = wpool.tile([2, BL], f32, tag="ab2")
            nc.vector.tensor_tensor(
                out=ab2[:], in0=pab[:], in1=E_abT[:, t * BL:(t + 1) * BL], op=ADD)
            nc.vector.tensor_tensor(
                out=ab2[:], in0=ab2[:], in1=gdT[:, t * BL:(t + 1) * BL], op=ADD)
            nc.vector.tensor_scalar(
                out=ab2[:], in0=ab2[:], scalar1=0.0, scalar2=None, op0=ISGT)
            # replicate f,i,o,alpha,beta across partitions: [128, (5,16)]
            prep = ps_p.tile([128, 5 * BL], f32, tag="ps")
            prep_pv = prep[:].rearrange("p (g b) -> p g b", g=5)
            for gidx in range(3):
                nc.tensor.matmul(
                    out=prep_pv[:, gidx, :],
                    lhsT=sel5[0:3, gidx * 128:(gidx + 1) * 128], rhs=gt3[:, :],
                    start=True, stop=True)
            for gidx in range(2):
                nc.tensor.matmul(
                    out=prep_pv[:, 3 + gidx, :],
                    lhsT=sel5[0:2, gidx * 128:(gidx + 1) * 128], rhs=ab2[:, :],
                    start=True, stop=True)
            rep_sb = wpool.tile([128, 5 * BL], f32, tag="repsb")
            nc.vector.tensor_copy(rep_sb[:], prep[:])
            prep_v = rep_sb[:].rearrange("p (g b) -> p g b", g=5)

            # ---- alpha-scaled r contributions -> pre_r psum [128,(8,16)]
            # alpha lives on partitions: scale Tg rows; and scale r_cT columns
            alphaT = wpool.tile([BL, 1], f32, tag="alphaT")
            # alpha as [16,1]: transpose row 3 of gab via PE
            pat = ps_p.tile([16, 2], f32, tag="ps")
            nc.tensor.transpose(out=pat[:], in_=ab2[:, :], identity=ident16[:2, :2])
            nc.vector.tensor_copy(alphaT[:], pat[:, 0:1])
            Tg_c = wpool.tile([BL, 1024], f32, tag="Tgc")
            nc.vector.tensor_scalar(
                out=Tg_c[:], in0=Tg[:, 0:1024], scalar1=alphaT[:, 0:1],
                scalar2=None, op0=MULT)
            rx = wpool.tile([128, BL], f32, tag="rx")
            nc.vector.tensor_tensor(
                out=rx[:], in0=r_cT[:], in1=prep_v[:, 3, :], op=MULT)
            pper = ps_p.tile([128, JH * BL], f32, tag="ps")
            pper_v = pper[:].rearrange("p (j b) -> p j b", j=JH)
            for m in range(JH):
                nc.tensor.matmul(
                    out=pper_v[:, m, :], lhsT=Wrc[:, m * 128:(m + 1) * 128],
                    rhs=rx[:], start=True, stop=False)
                nc.tensor.matmul(
                    out=pper_v[:, m, :], lhsT=Tg_c[:, m * 128:(m + 1) * 128],
                    rhs=ident16[:], start=False, stop=True)

            # ---- c_tilde, cc, h
            pre = wpool.tile([128, JH * BL], f32, tag="pre")
            pre_v = pre[:].rearrange("p (j b) -> p j b", j=JH)
            beta_bc = prep_v[:, 4:5, :].to_broadcast([128, JH, BL])
            nc.vector.tensor_tensor(out=pre_v, in0=phwc_v, in1=beta_bc, op=MULT)
            nc.vector.tensor_tensor(out=pre_v, in0=pre_v, in1=E_cT_v[:, :, t, :],
                                    op=ADD)
            nc.vector.tensor_tensor(out=pre_v, in0=pre_v, in1=pper_v, op=ADD)
            c_tilde = wpool.tile([128, JH * BL], f32, tag="ctl")
            nc.scalar.activation(c_tilde[:], pre[:], TANH)
            f_bc = prep_v[:, 0:1, :].to_broadcast([128, JH, BL])
            i_bc = prep_v[:, 1:2, :].to_broadcast([128, JH, BL])
            o_bc = prep_v[:, 2:3, :].to_broadcast([128, JH, BL])
            nc.vector.tensor_tensor(out=ccT_v, in0=ccT_v, in1=f_bc, op=MULT)
            ict = wpool.tile([128, JH * BL], f32, tag="ict")
            nc.vector.tensor_tensor(
                out=ict[:].rearrange("p (j b) -> p j b", j=JH),
                in0=c_tilde[:].rearrange("p (j b) -> p j b", j=JH),
                in1=i_bc, op=MULT)
            nc.vector.tensor_tensor(out=ccT[:], in0=ccT[:], in1=ict[:], op=ADD)
            tcc = wpool.tile([128, JH * BL], f32, tag="tcc")
            nc.scalar.activation(tcc[:], ccT[:], TANH)
            nc.vector.tensor_tensor(
                out=hT_v, in0=tcc[:].rearrange("p (j b) -> p j b", j=JH),
                in1=o_bc, op=MULT)

            # ---- emit output h_t
            nc.sync.dma_start(
                d_out[t].rearrange("(j p) b -> p j b", p=128),
                hT[:].rearrange("p (j b) -> p j b", j=JH))

            # ---- write side: val = h_new @ W_h2m  (feature-major [C,16])
            pval = ps_p.tile([128, BL], f32, tag="ps")
            for j in range(JH):
                nc.tensor.matmul(out=pval[:], lhsT=Whm_v[:, j, :], rhs=hT_v[:, j, :],
                                 start=(j == 0), stop=(j == JH - 1))
            valT = wpool.tile([128, BL], f32, tag="valT")
            nc.vector.tensor_copy(valT[:], pval[:])
            delta = wpool.tile([128, BL], f32, tag="delta")
            if t < 16:
                nc.vector.tensor_copy(delta[:], valT[:])
            else:
                nc.vector.tensor_tensor(out=delta[:], in0=valT[:], in1=r_cT[:],
                                        op=SUB)
            # M addend [k,16] per k-tile
            padd = [ps_p.tile([128, BL], f32, name=f"padd{_k}", tag="ps") for _k in range(KT)]
            for kt in range(KT):
                nc.tensor.matmul(
                    out=padd[kt][:], lhsT=Wm2A[:, kt * 128:(kt + 1) * 128],
                    rhs=delta[:], start=True, stop=True)
            # val row-major for C_mem scatter
            pvr = ps_p.tile([16, 128], f32, tag="ps")
            nc.tensor.transpose(out=pvr[:], in_=valT[:], identity=ident128[:])
            val_r = wpool.tile([BL, C], f32, tag="valr")
            nc.vector.tensor_copy(val_r[:], pvr[:])

            if t < 16:
                # static column 32t for every b: update M_sbuf directly
                for kt in range(KT):
                    Mv = M[kt][:].rearrange("p (b n) -> p b n", b=BL)
                    nc.vector.tensor_tensor(
                        out=Mv[:, :, 32 * t], in0=Mv[:, :, 32 * t],
                        in1=padd[kt][:], op=ADD)
                cm_vw = d_cmem[:].rearrange("(b n) c -> b n c", b=BL)
                nc.sync.dma_start(cm_vw[:, 32 * t, :], val_r[:])
            else:
                # append telescoping pair j-slice [(t-16)*16, +16):
                #   S_old = M0[pos] + r_c @ Wm2A (== current M column, frozen)
                #   S_new = S_old + delta @ Wm2A
                sl = slice((t - 16) * BL, (t - 16) * BL + BL)
                gM0 = wpool.tile([BL, AC], f32, tag="gM0")
                nc.gpsimd.indirect_dma_start(
                    out=gM0[:], out_offset=None, in_=d_M0rows[:],
                    in_offset=bass.IndirectOffsetOnAxis(ap=pos_i[:, :1], axis=0))
                for kt in range(KT):
                    pm0 = ps_p.tile([128, BL], f32, tag="ps")
                    nc.tensor.transpose(
                        out=pm0[:], in_=gM0[:, kt * 128:(kt + 1) * 128],
                        identity=ident16[:])
                    m0sb = wpool.tile([128, BL], f32, tag="m0sb")
                    nc.vector.tensor_copy(m0sb[:], pm0[:])
                    pAold = ps_p.tile([128, BL], f32, tag="ps")
                    nc.tensor.matmul(
                        out=pAold[:], lhsT=Wm2A[:, kt * 128:(kt + 1) * 128],
                        rhs=r_cT[:], start=True, stop=True)
                    nc.vector.tensor_tensor(
                        out=SoldT[kt][:, sl], in0=m0sb[:], in1=pAold[:], op=ADD)
                    nc.vector.tensor_tensor(
                        out=SnewT[kt][:, sl], in0=SoldT[kt][:, sl],
                        in1=padd[kt][:], op=ADD)
                # one-hot rows for this group (static partition slice)
                jrow = (t - 16) * BL
                ch, p0 = jrow // 128, jrow % 128
                oh_tmp = wpool.tile([BL, N], f32, tag="ohtmp")
                nc.vector.tensor_scalar(
                    out=oh_tmp[:], in0=iota16[:],
                    scalar1=pos_f[:, 0:1], scalar2=None, op0=ISEQ)
                nc.sync.dma_start(oneh[ch][p0:p0 + BL, :], oh_tmp[:])
                nc.gpsimd.indirect_dma_start(
                    out=d_cmem[:], in_=val_r[:], in_offset=None,
                    out_offset=bass.IndirectOffsetOnAxis(ap=cm_idx[:, :1], axis=0))

        # explicit ordering: first gather must follow zero-fill
        for gi_ in first_gather_insts:
            for zi in zfill:
                tile.add_dep_helper(gi_.ins, zi.ins, reason="C_mem zero before first gather")

    split_multi_waits(nc)
    return nc


# ---------------------------------------------------------------- host side

_PROG = None


def _get_program():
    global _PROG
    if _PROG is None:
        _PROG = build_program()
    return _PROG


def _sel5():
    s = np.zeros((5, 5, 128), np.float32)
    for g in range(5):
        s[g, g, :] = 1.0
    return s.reshape(5, 5 * 128)


def _host_prep(inputs):
    import jax
    import jax.numpy as jnp

    cpu = jax.devices("cpu")[0]
    with jax.default_device(cpu):
        rng = jax.random.key(42)
        k1, k2 = jax.random.split(rng)
        g_read = np.asarray(jax.random.gumbel(k1, (L, B, N), jnp.float32))
        g_ab = np.asarray(jax.random.gumbel(k2, (L, B, 2, 2), jnp.float32))
    gd = g_ab[:, :, :, 0] - g_ab[:, :, :, 1]  # [L,B,2]

    f32 = np.float32
    mem_bias = np.asarray(inputs["mem_bias"], f32)
    W_m2w = np.asarray(inputs["W_m2w"], f32)
    M0 = (mem_bias @ W_m2w).astype(f32)                      # [N, AC]
    addr = mem_bias[:, :A]
    T_cat = np.concatenate([
        addr @ np.asarray(inputs["W_r2c"], f32)[:A],
        addr @ np.asarray(inputs["W_r2gates"], f32)[:A],
        addr @ np.asarray(inputs["W_r2ab"], f32)[:A]], axis=1).astype(f32)  # [N,1029]
    colsum = np.asarray(inputs["W_u2w"], f32).sum(0).astype(f32)

    with jax.default_device(jax.devices("cpu")[0]):
        sqv = np.arange(1025, dtype=f32)
        lut = np.zeros((L + 1, 1025), f32)
        for t in range(L + 1):
            mu = f32(t) / f32(N)
            var = sqv / f32(N) - mu * mu
            lut[t] = np.asarray(jax.lax.rsqrt(jnp.asarray(var + f32(1e-5))))
    lut_flat = lut.reshape(-1, 1)

    def pack_kt(w):  # [HDIM, X] -> [128, JH*X]
        X = w.shape[1]
        return np.ascontiguousarray(
            w.reshape(JH, 128, X).transpose(1, 0, 2).reshape(128, JH * X))

    Whw = pack_kt(np.asarray(inputs["W_h2w"], f32))
    Whc = pack_kt(np.asarray(inputs["W_h2c"], f32))
    Whgab = pack_kt(np.concatenate(
        [np.asarray(inputs["W_h2gates"], f32),
         np.asarray(inputs["W_h2ab"], f32)], axis=1))
    Whm = pack_kt(np.asarray(inputs["W_h2m"], f32))
    Wrc = np.concatenate([
        np.asarray(inputs["W_r2c"], f32)[A:],
        np.asarray(inputs["W_r2gates"], f32)[A:],
        np.asarray(inputs["W_r2ab"], f32)[A:]], axis=1).astype(f32)  # [128,1029]
    Wm2A = np.ascontiguousarray(W_m2w[A:])                 # [128, 256]
    M0T = np.ascontiguousarray(
        M0.T.reshape(KT, 128, N).transpose(1, 0, 2).reshape(128, KT * N))
    base = np.asarray(inputs["atten_base"], f32)
    bmask = np.zeros((128, KT, BL, BL), f32)
    for kt in range(KT):
        for b in range(BL):
            bmask[:, kt, b, b] = base[kt * 128:(kt + 1) * 128]
    bmask = bmask.reshape(128, KT * BL * BL)
    Wicat = np.concatenate([
        np.asarray(inputs["W_i2w"], f32),
        np.asarray(inputs["W_i2c"], f32),
        np.asarray(inputs["W_i2gates"], f32),
        np.asarray(inputs["W_i2ab"], f32)], axis=1)  # [512, 1285]
    Wicat_p = np.ascontiguousarray(
        Wicat.reshape(JI, 128, 1285).transpose(1, 0, 2).reshape(128, JI * 1285))

    shared = {
        "Wicat": Wicat_p, "Whw": Whw, "Whc": Whc, "Whgab": Whgab,
        "Whm": Whm, "Wrc": Wrc, "Wm2A": Wm2A, "M0T": M0T, "bmask": bmask,
        "colsum16": np.tile(colsum[None, :], (BL, 1)),
        "iota16": np.tile(np.arange(N, dtype=f32)[None, :], (BL, 1)),
        "boff": (np.arange(BL, dtype=np.int32) * N)[:, None],
        "Tcat": T_cat, "Wu2w": np.asarray(inputs["W_u2w"], f32),
        "slut": lut_flat,
        "sel5": _sel5(),
        "M0rows": np.ascontiguousarray(M0),
        "base16": np.ascontiguousarray(
            np.repeat(base.reshape(KT, 128).transpose(1, 0)[:, :, None], BL, 2
                      ).reshape(128, KT * BL)),
        "bsel128": np.tile(np.eye(BL, dtype=f32), (8, 1)),
    }

    inp = np.asarray(inputs["inp"], f32)
    hid = np.asarray(inputs["hid"], f32)
    in_maps = []
    for core in range(NCORES):
        b0 = core * BL
        bsl = slice(b0, b0 + BL)
        # inpT [128, (JI, L*BL)]: inpT[p, j, l*BL+b] = inp[l, b0+b, j*128+p]
        it = inp[:, bsl, :].transpose(2, 0, 1).reshape(JI, 128, L * BL)
        inpT = np.ascontiguousarray(it.transpose(1, 0, 2).reshape(128, JI * L * BL))
        # h0T [128, (JH, BL)]
        h0 = hid[bsl].T.reshape(JH, 128, BL).transpose(1, 0, 2).reshape(128, JH * BL)
        g_core = np.ascontiguousarray(
            g_read[:, bsl, :].reshape(L * BL, N))
        gdT = np.ascontiguousarray(
            gd[:, bsl, :].transpose(2, 0, 1).reshape(2, L * BL))
        m = dict(shared)
        m.update({
            "inpT": inpT, "h0T": np.ascontiguousarray(h0),
            "g_read": g_core, "gdT": gdT,
        })
        in_maps.append(m)
    return in_maps


def kernel(**inputs) -> np.ndarray:
    nc = _get_program()
    in_maps = _host_prep(inputs)
    res = run_bass_kernel_spmd(nc, in_maps, core_ids=list(range(NCORES)))
    outs = np.zeros((L, B, HDIM), np.float32)
    for core in range(NCORES):
        o = res.results[core]["outs"]  # [L, HDIM, BL]
        outs[:, core * BL:(core + 1) * BL, :] = o.transpose(0, 2, 1)
    return outs


# revision 23
# speedup vs baseline: 1.4854x; 1.4854x over previous
"""TARDIS decoder Bass kernel for Trainium2, 8-way batch-parallel.

Strategy (per core, 16 batch elements):
- Keep M = mem @ W_m2w resident in SBUF as [k(2x128 part), (b=16, n=512) free];
  memory writes touch one slot per batch element per step, so M is updated
  incrementally (delta @ W_m2w[A:]) instead of recomputing the batched matmul.
- Hard gumbel-softmax == argmax(logits + g); tau/softplus drop out of the
  forward pass entirely.  Gumbel noise is reproduced host-side (fixed key 42).
- logits = sum_k base_k * tanh(c[b,k] + M[b,n,k]): tanh on ScalarE with the
  per-(b, k-tile) bias trick; the k-reduction runs on PE with masked-lhsT
  accumulation into one [16,512] PSUM tile.
- LayerNorm of w_sum reduces to an integer-indexed rsqrt LUT gather because
  w_sum is a sum of exact one-hots (mean = t/512 is a compile-time constant).
- Memory content lives in DRAM (C_mem) accessed via indirect row gather/
  scatter on the gpsimd dynamic queue; the address-bits contribution of every
  read comes from precomputed tables (addr @ W_r2*) gathered the same way.
"""

import contextlib
import ctypes
import os
import sys
import types

sys.path.insert(0, "/opt/trn_rl_repo")

import numpy as np

import bass_rust
import concourse.bass as bass
import concourse.tile as tile
from concourse import mybir
from concourse.bass_utils import run_bass_kernel_spmd
from concourse.masks import make_identity

dt = mybir.dt

L, B, IDIM, HDIM, N, A, C = 32, 128, 512, 1024, 512, 128, 128
AC = A + C
NCORES = 8
BL = B // NCORES  # 16
JH = HDIM // 128  # 8
KT = AC // 128    # 2
JI = IDIM // 128  # 4
NSTEPS = int(os.environ.get("TARDIS_STEPS", str(L)))


def _install_ntff_hook():
    """Register the axon NTFF profiling hook (missing antenv.axon_hooks shim)."""
    if "antenv.axon_hooks" in sys.modules:
        return
    so_path = "/opt/axon/libaxon_pjrt.so"
    try:
        lib = ctypes.CDLL(so_path)
        lib.axon_start_nrt_profile.argtypes = [
            ctypes.POINTER(ctypes.c_int64), ctypes.c_size_t]
        lib.axon_start_nrt_profile.restype = ctypes.c_int64
        lib.axon_stop_nrt_profile.argtypes = [ctypes.c_char_p]
        lib.axon_stop_nrt_profile.restype = ctypes.c_int64
    except OSError:
        return

    @contextlib.contextmanager
    def _hook(output_dir, device_ids):
        import jax
        jax.devices()
        if device_ids:
            ids = (ctypes.c_int64 * len(device_ids))(*device_ids)
            rc = lib.axon_start_nrt_profile(ids, len(device_ids))
        else:
            rc = lib.axon_start_nrt_profile(None, 0)
        if rc != 0:
            raise RuntimeError(f"axon_start_nrt_profile rc={rc}")
        try:
            yield
        finally:
            n = lib.axon_stop_nrt_profile(str(output_dir).encode())
            if n < 0:
                raise RuntimeError(f"axon_stop_nrt_profile rc={n}")

    mod = types.ModuleType("antenv.axon_hooks")
    mod.get_axon_ntff_profile_hook = lambda: _hook
    mod.set_axon_ntff_profile_hook = lambda h: None
    sys.modules["antenv.axon_hooks"] = mod


_install_ntff_hook()


def split_multi_waits(nc):
    """This container's walrus accepts only one sync-wait per instruction;
    hoist extra waits onto preceding NOPs on the same engine."""
    nsplit = 0
    for f in nc.m.functions:
        for blk in f.blocks:
            insts = blk.instructions
            newlist = []
            for inst in insts:
                si = inst.sync_info
                if si is not None and si.on_wait and len(si.on_wait) > 1:
                    waits = list(si.on_wait)
                    for w in waits[:-1]:
                        nop = mybir.InstNoOp(name=f"waitsplit_{nc.next_id()}")
                        nop.engine = inst.engine
                        nop.sync_info = bass_rust.SyncInfo(on_wait=[w], on_update=[])
                        newlist.append(nop)
                        nsplit += 1
                    si.on_wait = [waits[-1]]
                    inst.sync_info = si
                newlist.append(inst)
            insts[:] = newlist
    return nsplit


# ---------------------------------------------------------------- device code


def build_program():
    nc = bass.Bass(trn_type="TRN2")
    f32, i32, u32, f16 = dt.float32, dt.int32, dt.uint32, dt.float16
    TANH = mybir.ActivationFunctionType.Tanh
    SIG = mybir.ActivationFunctionType.Sigmoid
    ADD = mybir.AluOpType.add
    SUB = mybir.AluOpType.subtract
    MULT = mybir.AluOpType.mult
    ISEQ = mybir.AluOpType.is_equal
    ISGT = mybir.AluOpType.is_gt

    # -------- DRAM I/O (names = in_map keys)
    d_inpT = nc.dram_tensor("inpT", [128, JI * L * BL], f32, kind="ExternalInput")
    d_h0T = nc.dram_tensor("h0T", [128, JH * BL], f32, kind="ExternalInput")
    d_Wicat = nc.dram_tensor("Wicat", [128, JI * 1285], f32, kind="ExternalInput")
    d_Whw = nc.dram_tensor("Whw", [128, JH * AC], f32, kind="ExternalInput")
    d_Whc = nc.dram_tensor("Whc", [128, JH * HDIM], f16, kind="ExternalInput")
    d_Whgab = nc.dram_tensor("Whgab", [128, JH * 5], f32, kind="ExternalInput")
    d_Whm = nc.dram_tensor("Whm", [128, JH * C], f16, kind="ExternalInput")
    d_Wrc = nc.dram_tensor("Wrc", [128, 1029], f32, kind="ExternalInput")
    d_Wrc16 = nc.dram_tensor("Wrc16", [128, 1024], f16, kind="ExternalInput")
    d_Wm2A = nc.dram_tensor("Wm2A", [128, AC], f32, kind="ExternalInput")
    d_M0T = nc.dram_tensor("M0T", [128, KT * N], f32, kind="ExternalInput")
    d_bmask = nc.dram_tensor("bmask", [128, KT * BL * BL], f16, kind="ExternalInput")
    d_colsum = nc.dram_tensor("colsum16", [BL, AC], f32, kind="ExternalInput")
    d_iota = nc.dram_tensor("iota16", [BL, N], f32, kind="ExternalInput")
    d_boff = nc.dram_tensor("boff", [BL, 1], i32, kind="ExternalInput")
    d_g = nc.dram_tensor("g_read", [L * BL, N], f32, kind="ExternalInput")
    d_gdT = nc.dram_tensor("gdT", [2, L * BL], f32, kind="ExternalInput")
    d_Tcat = nc.dram_tensor("Tcat", [N, 1029], f32, kind="ExternalInput")
    d_Wu2w = nc.dram_tensor("Wu2w", [N, AC], f32, kind="ExternalInput")
    d_lut = nc.dram_tensor("slut", [(L + 1) * 1025, 1], f32, kind="ExternalInput")
    d_sel5 = nc.dram_tensor("sel5", [5, 5 * 128], f32, kind="ExternalInput")
    d_M0rows = nc.dram_tensor("M0rows", [N, AC], f32, kind="ExternalInput")
    d_base16 = nc.dram_tensor("base16", [128, KT * BL], f16, kind="ExternalInput")
    d_bsel = nc.dram_tensor("bsel128", [128, BL], f32, kind="ExternalInput")

    d_out = nc.dram_tensor("outs", [L, HDIM, BL], f32, kind="ExternalOutput")

    d_cmem = nc.dram_tensor("C_mem", [BL * N, C], f32, kind="Internal")

    with tile.TileContext(nc) as tc, contextlib.ExitStack() as ctx:
        cpool = ctx.enter_context(tc.tile_pool(name="const", bufs=1))
        spool = ctx.enter_context(tc.tile_pool(name="state", bufs=1))

        # ---------------- persistent constants
        Whw = cpool.tile([128, JH * AC], f32)
        Whc = cpool.tile([128, JH * HDIM], f16)
        Whgab = cpool.tile([128, JH * 5], f32)
        Whm = cpool.tile([128, JH * C], f16)
        Wrc = cpool.tile([128, 1029], f32)
        Wrc16 = cpool.tile([128, 1024], f16)
        Wm2A = cpool.tile([128, AC], f32)
        bmask = cpool.tile([128, KT * BL * BL], f16)
        colsum16 = cpool.tile([BL, AC], f32)
        iota16 = cpool.tile([BL, N], f32)
        boff = cpool.tile([BL, 1], i32)
        gdT = cpool.tile([2, L * BL], f32)
        sel5 = cpool.tile([5, 5 * 128], f32)
        base16 = cpool.tile([128, KT * BL], f16)
        bsel128 = cpool.tile([128, BL], f32)
        ident16 = cpool.tile([16, 16], f32)
        ident128 = cpool.tile([128, 128], f32)
        E_wT = cpool.tile([128, KT * L * BL], f32)     # 4 KB/part
        E_cT = cpool.tile([128, JH * L * BL], f32)     # 16 KB/part
        E_gT = cpool.tile([3, L * BL], f32)
        E_abT = cpool.tile([2, L * BL], f32)

        for t_, d_ in [(Whw, d_Whw), (Whc, d_Whc), (Whgab, d_Whgab),
                       (Whm, d_Whm), (Wrc, d_Wrc), (Wm2A, d_Wm2A),
                       (bmask, d_bmask), (colsum16, d_colsum),
                       (iota16, d_iota), (boff, d_boff), (gdT, d_gdT),
                       (sel5, d_sel5), (base16, d_base16), (bsel128, d_bsel),
                       (Wrc16, d_Wrc16)]:
            nc.sync.dma_start(t_[:], d_[:])
        make_identity(nc, ident16[:])
        make_identity(nc, ident128[:])

        # ---------------- persistent state
        hT = spool.tile([128, JH * BL], f32)       # h^T  [128,(8,16)]
        ccT = spool.tile([128, JH * BL], f32)      # cc^T
        M = [spool.tile([128, BL * N], f32, name=f"M{_k}") for _k in range(KT)]  # 32 KB/part each
        U = spool.tile([BL, AC], f32)
        w_sum = spool.tile([BL, N], f32)
        sq = spool.tile([BL, 1], f32)
        s_cur = spool.tile([BL, 1], f32)
        # telescoping-correction pair state: columns j = (t_w-16)*16 + b
        NPAIR = (L - 16) * BL  # 256
        SoldT = [spool.tile([128, NPAIR], f32, name=f"SoldT{_k}") for _k in range(KT)]
        SnewT = [spool.tile([128, NPAIR], f32, name=f"SnewT{_k}") for _k in range(KT)]
        oneh = [spool.tile([128, N], f16, name=f"oneh{_k}") for _k in range(NPAIR // 128)]

        nc.sync.dma_start(hT[:], d_h0T[:])
        nc.vector.memset(ccT[:], 0.0)
        nc.vector.memset(U[:], 0.0)
        nc.vector.memset(w_sum[:], 0.0)
        nc.vector.memset(sq[:], 0.0)
        nc.vector.memset(s_cur[:], float(np.float32(1.0) / np.sqrt(np.float32(1e-5))))

        # M init: M[kt][:, b*N:(b+1)*N] = M0T[:, kt*N:(kt+1)*N] for each b
        for kt in range(KT):
            for b in range(BL):
                nc.sync.dma_start(M[kt][:, b * N:(b + 1) * N],
                                  d_M0T[:, kt * N:(kt + 1) * N])

        # C_mem zero-fill (16 x 512 rows), keep insts for an explicit dep
        zfill = []
        with tc.tile_pool(name="zt", bufs=1) as zpool:
            zt = zpool.tile([128, 512], f32)
            nc.vector.memset(zt[:], 0.0)
            for a in range(BL):
                cm_v = d_cmem[a * 512:(a + 1) * 512, :].rearrange(
                    "(p q) c -> p (q c)", p=128)
                zfill.append(nc.sync.dma_start(cm_v, zt[:]))

        # ---------------- E-precompute: E_*T = (inp @ W_i*)^T, feature-major
        with tc.tile_pool(name="pre", bufs=1) as prepool, \
             tc.tile_pool(name="prepsum", bufs=4, space="PSUM") as prepsum:
            inpT = prepool.tile([128, JI * L * BL], f32)
            Wicat = prepool.tile([128, JI * 1285], f32)
            nc.sync.dma_start(inpT[:], d_inpT[:])
            nc.sync.dma_start(Wicat[:], d_Wicat[:])
            inpT_v = inpT[:].rearrange("p (j lb) -> p j lb", j=JI)
            Wic_v = Wicat[:].rearrange("p (j m) -> p j m", j=JI)
            # feature tiles: 0..1 -> E_wT, 2..9 -> E_cT, then gates [3], ab [2]
            for mt in range(12):
                if mt < 10:
                    m0, mw = mt * 128, 128
                elif mt == 10:
                    m0, mw = 1280, 3
                else:
                    m0, mw = 1283, 2
                pe = prepsum.tile([128, L * BL], f32, tag="pe")
                for j in range(JI):
                    nc.tensor.matmul(
                        out=pe[:mw, :], lhsT=Wic_v[:, j, m0:m0 + mw],
                        rhs=inpT_v[:, j, :], start=(j == 0), stop=(j == JI - 1))
                if mt < 2:
                    nc.vector.tensor_copy(
                        E_wT[:].rearrange("p (k lb) -> p k lb", k=KT)[:, mt, :],
                        pe[:, :])
                elif mt < 10:
                    nc.vector.tensor_copy(
                        E_cT[:].rearrange("p (j lb) -> p j lb", j=JH)[:, mt - 2, :],
                        pe[:, :])
                elif mt == 10:
                    nc.vector.tensor_copy(E_gT[:], pe[:3, :])
                else:
                    nc.vector.tensor_copy(E_abT[:], pe[:2, :])

        # ---------------- main pools
        wpool = ctx.enter_context(tc.tile_pool(name="work", bufs=1))
        tpool = ctx.enter_context(tc.tile_pool(name="tanh", bufs=4))
        gpool = ctx.enter_context(tc.tile_pool(name="gbuf", bufs=2))
        plog_p = ctx.enter_context(tc.tile_pool(name="plog", bufs=2, space="PSUM"))
        ps_p = ctx.enter_context(tc.tile_pool(name="ps", bufs=4, space="PSUM"))

        hT_v = hT[:].rearrange("p (j b) -> p j b", j=JH)
        ccT_v = ccT[:].rearrange("p (j b) -> p j b", j=JH)
        Whw_v = Whw[:].rearrange("p (j m) -> p j m", j=JH)
        Whc_v = Whc[:].rearrange("p (j m) -> p j m", j=JH)
        Whgab_v = Whgab[:].rearrange("p (j m) -> p j m", j=JH)
        Whm_v = Whm[:].rearrange("p (j m) -> p j m", j=JH)
        E_wT_v = E_wT[:].rearrange("p (k l b) -> p k l b", k=KT, l=L)
        E_cT_v = E_cT[:].rearrange("p (j l b) -> p j l b", j=JH, l=L)
        bmask_v = bmask[:].rearrange("p (k b m) -> p k b m", k=KT, b=BL)

        first_gather_insts = []

        for t in range(NSTEPS):
            mu_t = np.float32(t) / np.float32(N)

            # fp16 copy of h^T for the fp16 matmuls
            hT16p = wpool.tile([128, JH * BL], f16, tag="hT16p")
            nc.vector.tensor_copy(hT16p[:], hT[:])
            hT16p_v = hT16p[:].rearrange("p (j b) -> p j b", j=JH)

            # ---- cT = (h @ W_h2w)^T + E_wT[t] + diag(s)-terms   [k, b]
            Um = wpool.tile([BL, AC], f32, tag="Um")
            # U_minus = U - mu_t * colsum  (mu_0 = 0 but keep uniform)
            nc.vector.tensor_scalar(
                out=Um[:], in0=colsum16[:], scalar1=-float(mu_t), scalar2=None,
                op0=MULT)
            nc.vector.tensor_tensor(out=Um[:], in0=Um[:], in1=U[:], op=ADD)
            diag_s = wpool.tile([BL, BL], f32, tag="diag")
            nc.vector.tensor_scalar(
                out=diag_s[:], in0=ident16[:], scalar1=s_cur[:, 0:1], scalar2=None,
                op0=MULT)
            cT = [wpool.tile([128, BL], f32, name=f"cT{_k}", tag=f"cT{_k}") for _k in range(KT)]
            for kt in range(KT):
                pc = ps_p.tile([128, BL], f32, tag="ps")
                for j in range(JH):
                    nc.tensor.matmul(
                        out=pc[:], lhsT=Whw_v[:, j, kt * 128:(kt + 1) * 128],
                        rhs=hT_v[:, j, :], start=(j == 0), stop=False)
                nc.tensor.matmul(
                    out=pc[:], lhsT=Um[:, kt * 128:(kt + 1) * 128], rhs=diag_s[:],
                    start=False, stop=True)
                nc.vector.tensor_tensor(
                    out=cT[kt][:], in0=pc[:], in1=E_wT_v[:, kt, t, :], op=ADD)

            # ---- big tanh + masked-lhsT logits accumulation
            J = max(0, (t - 16)) * BL  # live correction pairs
            n_corr_mm = (J + 127) // 128
            plog = plog_p.tile([BL, N], f32, tag="plog")
            n_mm = 0
            for b in range(BL):
                for kt in range(KT):
                    th = tpool.tile([128, N], f16, tag="th")
                    nc.scalar.activation(
                        th[:], M[kt][:, b * N:(b + 1) * N], TANH,
                        bias=cT[kt][:, b:b + 1], scale=1.0)
                    nc.tensor.matmul(
                        out=plog[:], lhsT=bmask_v[:, kt, b, :], rhs=th[:],
                        start=(n_mm == 0),
                        stop=(n_mm == 2 * BL - 1 and n_corr_mm == 0))
                    n_mm += 1

            # ---- telescoping corrections for post-step-16 writes
            if J > 0:
                G = J // BL
                dts = []
                for kt in range(KT):
                    cbc = cT[kt][:].rearrange(
                        "p (one b) -> p one b", one=1).to_broadcast([128, G, BL])
                    argn = wpool.tile([128, NPAIR], f32, tag=f"argn{kt}")
                    argo = wpool.tile([128, NPAIR], f32, tag=f"argo{kt}")
                    dtn = wpool.tile([128, NPAIR], f16, tag=f"dtn{kt}")
                    argn_v = argn[:, :J].rearrange("p (g b) -> p g b", g=G)
                    argo_v = argo[:, :J].rearrange("p (g b) -> p g b", g=G)
                    nc.vector.tensor_tensor(
                        out=argn_v, in0=SnewT[kt][:, :J].rearrange(
                            "p (g b) -> p g b", g=G), in1=cbc, op=ADD)
                    nc.vector.tensor_tensor(
                        out=argo_v, in0=SoldT[kt][:, :J].rearrange(
                            "p (g b) -> p g b", g=G), in1=cbc, op=ADD)
                    nc.scalar.activation(argn[:, :J], argn[:, :J], TANH)
                    nc.scalar.activation(argo[:, :J], argo[:, :J], TANH)
                    nc.vector.tensor_tensor(
                        out=dtn[:, :J], in0=argn[:, :J], in1=argo[:, :J], op=SUB)
                    dts.append(dtn)
                pout1 = ps_p.tile([BL, NPAIR], f32, tag="ps")
                for kt in range(KT):
                    nc.tensor.matmul(
                        out=pout1[:, :J], lhsT=base16[:, kt * BL:(kt + 1) * BL],
                        rhs=dts[kt][:, :J], start=(kt == 0), stop=(kt == KT - 1))
                out1_sb = wpool.tile([BL, NPAIR], f32, tag="out1sb")
                nc.vector.tensor_copy(out1_sb[:, :J], pout1[:, :J])
                for ch in range(n_corr_mm):
                    chJ = min(128, J - ch * 128)
                    ptc = ps_p.tile([128, BL], f32, tag="ps")
                    nc.tensor.transpose(
                        out=ptc[:chJ, :],
                        in_=out1_sb[:, ch * 128:ch * 128 + chJ], identity=ident16[:])
                    lhsT2 = wpool.tile([128, BL], f16, tag="lhsT2")
                    nc.vector.tensor_tensor(
                        out=lhsT2[:chJ, :], in0=ptc[:chJ, :], in1=bsel128[:chJ, :],
                        op=MULT)
                    nc.tensor.matmul(
                        out=plog[:], lhsT=lhsT2[:chJ, :], rhs=oneh[ch][:chJ, :],
                        start=False, stop=(ch == n_corr_mm - 1))

            # ---- h @ W_h2c during the tanh window (no dep on logits)
            # b-major fp16: [16, 1024] psum, then transpose to [128,(8,16)]
            phwcB = ps_p.tile([16, HDIM], f32, tag="pwide", bufs=1)
            for ch in range(2):
                for j in range(JH):
                    nc.tensor.matmul(
                        out=phwcB[:, ch * 512:(ch + 1) * 512],
                        lhsT=hT16p_v[:, j, :],
                        rhs=Whc_v[:, j, ch * 512:(ch + 1) * 512],
                        start=(j == 0), stop=(j == JH - 1))
            hwcB = wpool.tile([16, HDIM], f32, tag="hwcB")
            nc.scalar.copy(hwcB[:], phwcB[:])
            phwc = ps_p.tile([128, JH * BL], f32, tag="ps")
            phwc_v = phwc[:].rearrange("p (j b) -> p j b", j=JH)
            for m in range(JH):
                nc.tensor.transpose(
                    out=phwc_v[:, m, :], in_=hwcB[:, m * 128:(m + 1) * 128],
                    identity=ident16[:])
            # h @ W_h2gates -> [3, b]; h @ W_h2ab -> [2, b]
            pgt = ps_p.tile([3, BL], f32, tag="ps")
            pab = ps_p.tile([2, BL], f32, tag="ps")
            for j in range(JH):
                nc.tensor.matmul(
                    out=pgt[:], lhsT=Whgab_v[:, j, 0:3], rhs=hT_v[:, j, :],
                    start=(j == 0), stop=False)
                nc.tensor.matmul(
                    out=pab[:], lhsT=Whgab_v[:, j, 3:5], rhs=hT_v[:, j, :],
                    start=(j == 0), stop=False)

            # ---- logits + g, argmax
            g_t = gpool.tile([BL, N], f32, tag="g")
            nc.sync.dma_start(g_t[:], d_g[t * BL:(t + 1) * BL, :])
            logits = wpool.tile([BL, N], f32, tag="logits")
            nc.vector.tensor_tensor(out=logits[:], in0=plog[:], in1=g_t[:], op=ADD)
            mx8 = wpool.tile([BL, 8], f32, tag="mx8")
            mi8 = wpool.tile([BL, 8], u32, tag="mi8")
            nc.vector.max(out=mx8[:], in_=logits[:])
            nc.vector.max_index(out=mi8[:], in_max=mx8[:], in_values=logits[:])
            pos_i = wpool.tile([BL, 1], i32, tag="posi")
            nc.vector.tensor_copy(pos_i[:], mi8[:, 0:1])
            pos_f = wpool.tile([BL, 1], f32, tag="posf")
            nc.vector.tensor_copy(pos_f[:], mi8[:, 0:1])

            # ---- one-hot w, v, state updates (read side)
            w = wpool.tile([BL, N], f32, tag="w")
            nc.vector.tensor_scalar(
                out=w[:], in0=iota16[:], scalar1=pos_f[:, 0:1], scalar2=None,
                op0=ISEQ)
            wwsum = wpool.tile([BL, N], f32, tag="wwsum")
            v = wpool.tile([BL, 1], f32, tag="v")
            nc.vector.tensor_tensor(out=wwsum[:], in0=w[:], in1=w_sum[:], op=MULT)
            nc.vector.tensor_reduce(
                out=v[:], in_=wwsum[:], axis=mybir.AxisListType.X, op=ADD)
            nc.vector.tensor_tensor(out=w_sum[:], in0=w_sum[:], in1=w[:], op=ADD)
            # sq += 2v + 1 ; lut index = sq + (t+1)*1025 (values are exact ints)
            v21 = wpool.tile([BL, 1], f32, tag="v21")
            nc.vector.tensor_scalar(
                out=v21[:], in0=v[:], scalar1=2.0, scalar2=1.0, op0=MULT, op1=ADD)
            nc.vector.tensor_tensor(out=sq[:], in0=sq[:], in1=v21[:], op=ADD)
            lut_if = wpool.tile([BL, 1], f32, tag="lutif")
            nc.vector.tensor_scalar(
                out=lut_if[:], in0=sq[:], scalar1=float((t + 1) * 1025) + 0.5,
                scalar2=None, op0=ADD)
            lut_ii = wpool.tile([BL, 1], i32, tag="lutii")
            nc.vector.tensor_copy(lut_ii[:], lut_if[:])
            gi = nc.gpsimd.indirect_dma_start(
                out=s_cur[:], out_offset=None, in_=d_lut[:],
                in_offset=bass.IndirectOffsetOnAxis(ap=lut_ii[:, :1], axis=0))

            # ---- gathers at pos
            cm_idx = wpool.tile([BL, 1], i32, tag="cmidx")
            nc.vector.tensor_tensor(out=cm_idx[:], in0=pos_i[:], in1=boff[:], op=ADD)
            r_c = wpool.tile([BL, C], f32, tag="rc")
            g1 = nc.gpsimd.indirect_dma_start(
                out=r_c[:], out_offset=None, in_=d_cmem[:],
                in_offset=bass.IndirectOffsetOnAxis(ap=cm_idx[:, :1], axis=0))
            if t == 0:
                first_gather_insts.append(g1)
            Tg = wpool.tile([BL, 1029], f32, tag="Tg")
            nc.gpsimd.indirect_dma_start(
                out=Tg[:], out_offset=None, in_=d_Tcat[:],
                in_offset=bass.IndirectOffsetOnAxis(ap=pos_i[:, :1], axis=0))
            wu = wpool.tile([BL, AC], f32, tag="wu")
            nc.gpsimd.indirect_dma_start(
                out=wu[:], out_offset=None, in_=d_Wu2w[:],
                in_offset=bass.IndirectOffsetOnAxis(ap=pos_i[:, :1], axis=0))
            nc.vector.tensor_tensor(out=U[:], in0=U[:], in1=wu[:], op=ADD)

            # ---- r_c^T via PE transpose
            ptr = ps_p.tile([128, BL], f32, tag="ps")
            nc.tensor.transpose(out=ptr[:], in_=r_c[:], identity=ident16[:])
            r_cT = wpool.tile([128, BL], f32, tag="rcT")
            nc.vector.tensor_copy(r_cT[:], ptr[:])

            # ---- gates / ab  (feature-major, base-0 tiles)
            nc.tensor.matmul(out=pgt[:], lhsT=Wrc[:, 1024:1027], rhs=r_cT[:],
                             start=False, stop=False)
            nc.tensor.matmul(out=pgt[:], lhsT=Tg[:, 1024:1027], rhs=ident16[:],
                             start=False, stop=True)
            nc.tensor.matmul(out=pab[:], lhsT=Wrc[:, 1027:1029], rhs=r_cT[:],
                             start=False, stop=False)
            nc.tensor.matmul(out=pab[:], lhsT=Tg[:, 1027:1029], rhs=ident16[:],
                             start=False, stop=True)
            gt3 = wpool.tile([3, BL], f32, tag="gt3")
            nc.vector.tensor_tensor(
                out=gt3[:], in0=pgt[:], in1=E_gT[:, t * BL:(t + 1) * BL], op=ADD)
            nc.scalar.activation(gt3[:], gt3[:], SIG)
            ab2 = wpool.tile([2, BL], f32, tag="ab2")
            nc.vector.tensor_tensor(
                out=ab2[:], in0=pab[:], in1=E_abT[:, t * BL:(t + 1) * BL], op=ADD)
            nc.vector.tensor_tensor(
                out=ab2[:], in0=ab2[:], in1=gdT[:, t * BL:(t + 1) * BL], op=ADD)
            nc.vector.tensor_scalar(
                out=ab2[:], in0=ab2[:], scalar1=0.0, scalar2=None, op0=ISGT)
            # replicate f,i,o,alpha,beta across partitions: [128, (5,16)]
            prep = ps_p.tile([128, 5 * BL], f32, tag="ps")
            prep_pv = prep[:].rearrange("p (g b) -> p g b", g=5)
            for gidx in range(3):
                nc.tensor.matmul(
                    out=prep_pv[:, gidx, :],
                    lhsT=sel5[0:3, gidx * 128:(gidx + 1) * 128], rhs=gt3[:, :],
                    start=True, stop=True)
            for gidx in range(2):
                nc.tensor.matmul(
                    out=prep_pv[:, 3 + gidx, :],
                    lhsT=sel5[0:2, gidx * 128:(gidx + 1) * 128], rhs=ab2[:, :],
                    start=True, stop=True)
            rep_sb = wpool.tile([128, 5 * BL], f32, tag="repsb")
            nc.vector.tensor_copy(rep_sb[:], prep[:])
            prep_v = rep_sb[:].rearrange("p (g b) -> p g b", g=5)

            # ---- alpha-scaled r contributions -> pre_r psum [128,(8,16)]
            # alpha lives on partitions: scale Tg rows; and scale r_cT columns
            alphaT = wpool.tile([BL, 1], f32, tag="alphaT")
            # alpha as [16,1]: transpose row 3 of gab via PE
            pat = ps_p.tile([16, 2], f32, tag="ps")
            nc.tensor.transpose(out=pat[:], in_=ab2[:, :], identity=ident16[:2, :2])
            nc.vector.tensor_copy(alphaT[:], pat[:, 0:1])
            Tg_c = wpool.tile([BL, 1024], f32, tag="Tgc")
            nc.vector.tensor_scalar(
                out=Tg_c[:], in0=Tg[:, 0:1024], scalar1=alphaT[:, 0:1],
                scalar2=None, op0=MULT)
            rx = wpool.tile([128, BL], f16, tag="rx")
            nc.vector.tensor_tensor(
                out=rx[:], in0=r_cT[:], in1=prep_v[:, 3, :], op=MULT)
            pper = ps_p.tile([128, JH * BL], f32, tag="ps")
            pper_v = pper[:].rearrange("p (j b) -> p j b", j=JH)
            for m in range(JH):
                nc.tensor.matmul(
                    out=pper_v[:, m, :], lhsT=Wrc16[:, m * 128:(m + 1) * 128],
                    rhs=rx[:], start=True, stop=False)
                nc.tensor.matmul(
                    out=pper_v[:, m, :], lhsT=Tg_c[:, m * 128:(m + 1) * 128],
                    rhs=ident16[:], start=False, stop=True)

            # ---- c_tilde, cc, h
            pre = wpool.tile([128, JH * BL], f32, tag="pre")
            pre_v = pre[:].rearrange("p (j b) -> p j b", j=JH)
            beta_bc = prep_v[:, 4:5, :].to_broadcast([128, JH, BL])
            nc.vector.tensor_tensor(out=pre_v, in0=phwc_v, in1=beta_bc, op=MULT)
            nc.vector.tensor_tensor(out=pre_v, in0=pre_v, in1=E_cT_v[:, :, t, :],
                                    op=ADD)
            nc.vector.tensor_tensor(out=pre_v, in0=pre_v, in1=pper_v, op=ADD)
            c_tilde = wpool.tile([128, JH * BL], f32, tag="ctl")
            nc.scalar.activation(c_tilde[:], pre[:], TANH)
            f_bc = prep_v[:, 0:1, :].to_broadcast([128, JH, BL])
            i_bc = prep_v[:, 1:2, :].to_broadcast([128, JH, BL])
            o_bc = prep_v[:, 2:3, :].to_broadcast([128, JH, BL])
            nc.vector.tensor_tensor(out=ccT_v, in0=ccT_v, in1=f_bc, op=MULT)
            ict = wpool.tile([128, JH * BL], f32, tag="ict")
            nc.vector.tensor_tensor(
                out=ict[:].rearrange("p (j b) -> p j b", j=JH),
                in0=c_tilde[:].rearrange("p (j b) -> p j b", j=JH),
                in1=i_bc, op=MULT)
            nc.vector.tensor_tensor(out=ccT[:], in0=ccT[:], in1=ict[:], op=ADD)
            tcc = wpool.tile([128, JH * BL], f32, tag="tcc")
            nc.scalar.activation(tcc[:], ccT[:], TANH)
            nc.vector.tensor_tensor(
                out=hT_v, in0=tcc[:].rearrange("p (j b) -> p j b", j=JH),
                in1=o_bc, op=MULT)

            # ---- emit output h_t
            nc.sync.dma_start(
                d_out[t].rearrange("(j p) b -> p j b", p=128),
                hT[:].rearrange("p (j b) -> p j b", j=JH))

            # ---- write side: val = h_new @ W_h2m  (b-major fp16)
            hT16n = wpool.tile([128, JH * BL], f16, tag="hT16n")
            nc.vector.tensor_copy(hT16n[:], hT[:])
            hT16n_v = hT16n[:].rearrange("p (j b) -> p j b", j=JH)
            pvalB = ps_p.tile([BL, C], f32, tag="ps")
            for j in range(JH):
                nc.tensor.matmul(out=pvalB[:], lhsT=hT16n_v[:, j, :],
                                 rhs=Whm_v[:, j, :],
                                 start=(j == 0), stop=(j == JH - 1))
            val_r = wpool.tile([BL, C], f32, tag="valr")
            nc.vector.tensor_copy(val_r[:], pvalB[:])
            delta_b = wpool.tile([BL, C], f32, tag="deltab")
            if t < 16:
                nc.vector.tensor_copy(delta_b[:], val_r[:])
            else:
                nc.vector.tensor_tensor(out=delta_b[:], in0=val_r[:], in1=r_c[:],
                                        op=SUB)
            pdT = ps_p.tile([128, BL], f32, tag="ps")
            nc.tensor.transpose(out=pdT[:], in_=delta_b[:], identity=ident16[:])
            deltaT = wpool.tile([128, BL], f32, tag="deltaT")
            nc.vector.tensor_copy(deltaT[:], pdT[:])
            # M addend [k,16] per k-tile
            padd = [ps_p.tile([128, BL], f32, name=f"padd{_k}", tag="ps") for _k in range(KT)]
            for kt in range(KT):
                nc.tensor.matmul(
                    out=padd[kt][:], lhsT=Wm2A[:, kt * 128:(kt + 1) * 128],
                    rhs=deltaT[:], start=True, stop=True)

            if t < 16:
                # static column 32t for every b: update M_sbuf directly
                for kt in range(KT):
                    Mv = M[kt][:].rearrange("p (b n) -> p b n", b=BL)
                    nc.vector.tensor_tensor(
                        out=Mv[:, :, 32 * t], in0=Mv[:, :, 32 * t],
                        in1=padd[kt][:], op=ADD)
                cm_vw = d_cmem[:].rearrange("(b n) c -> b n c", b=BL)
                nc.sync.dma_start(cm_vw[:, 32 * t, :], val_r[:])
            else:
                # append telescoping pair j-slice [(t-16)*16, +16):
                #   S_old = M0[pos] + r_c @ Wm2A (== current M column, frozen)
                #   S_new = S_old + delta @ Wm2A
                sl = slice((t - 16) * BL, (t - 16) * BL + BL)
                gM0 = wpool.tile([BL, AC], f32, tag="gM0")
                nc.gpsimd.indirect_dma_start(
                    out=gM0[:], out_offset=None, in_=d_M0rows[:],
                    in_offset=bass.IndirectOffsetOnAxis(ap=pos_i[:, :1], axis=0))
                for kt in range(KT):
                    pm0 = ps_p.tile([128, BL], f32, tag="ps")
                    nc.tensor.transpose(
                        out=pm0[:], in_=gM0[:, kt * 128:(kt + 1) * 128],
                        identity=ident16[:])
                    m0sb = wpool.tile([128, BL], f32, tag="m0sb")
                    nc.vector.tensor_copy(m0sb[:], pm0[:])
                    pAold = ps_p.tile([128, BL], f32, tag="ps")
                    nc.tensor.matmul(
                        out=pAold[:], lhsT=Wm2A[:, kt * 128:(kt + 1) * 128],
                        rhs=r_cT[:], start=True, stop=True)
                    nc.vector.tensor_tensor(
                        out=SoldT[kt][:, sl], in0=m0sb[:], in1=pAold[:], op=ADD)
                    nc.vector.tensor_tensor(
                        out=SnewT[kt][:, sl], in0=SoldT[kt][:, sl],
                        in1=padd[kt][:], op=ADD)
                # one-hot rows for this group (static partition slice)
                jrow = (t - 16) * BL
                ch, p0 = jrow // 128, jrow % 128
                oh_tmp = wpool.tile([BL, N], f16, tag="ohtmp")
                nc.vector.tensor_scalar(
                    out=oh_tmp[:], in0=iota16[:],
                    scalar1=pos_f[:, 0:1], scalar2=None, op0=ISEQ)
                nc.sync.dma_start(oneh[ch][p0:p0 + BL, :], oh_tmp[:])
                nc.gpsimd.indirect_dma_start(
                    out=d_cmem[:], in_=val_r[:], in_offset=None,
                    out_offset=bass.IndirectOffsetOnAxis(ap=cm_idx[:, :1], axis=0))

        # explicit ordering: first gather must follow zero-fill
        for gi_ in first_gather_insts:
            for zi in zfill:
                tile.add_dep_helper(gi_.ins, zi.ins, reason="C_mem zero before first gather")

    split_multi_waits(nc)
    return nc


# ---------------------------------------------------------------- host side

_PROG = None


def _get_program():
    global _PROG
    if _PROG is None:
        _PROG = build_program()
    return _PROG


def _sel5():
    s = np.zeros((5, 5, 128), np.float32)
    for g in range(5):
        s[g, g, :] = 1.0
    return s.reshape(5, 5 * 128)


def _host_prep(inputs):
    import jax
    import jax.numpy as jnp

    cpu = jax.devices("cpu")[0]
    with jax.default_device(cpu):
        rng = jax.random.key(42)
        k1, k2 = jax.random.split(rng)
        g_read = np.asarray(jax.random.gumbel(k1, (L, B, N), jnp.float32))
        g_ab = np.asarray(jax.random.gumbel(k2, (L, B, 2, 2), jnp.float32))
    gd = g_ab[:, :, :, 0] - g_ab[:, :, :, 1]  # [L,B,2]

    f32 = np.float32
    mem_bias = np.asarray(inputs["mem_bias"], f32)
    W_m2w = np.asarray(inputs["W_m2w"], f32)
    M0 = (mem_bias @ W_m2w).astype(f32)                      # [N, AC]
    addr = mem_bias[:, :A]
    T_cat = np.concatenate([
        addr @ np.asarray(inputs["W_r2c"], f32)[:A],
        addr @ np.asarray(inputs["W_r2gates"], f32)[:A],
        addr @ np.asarray(inputs["W_r2ab"], f32)[:A]], axis=1).astype(f32)  # [N,1029]
    colsum = np.asarray(inputs["W_u2w"], f32).sum(0).astype(f32)

    with jax.default_device(jax.devices("cpu")[0]):
        sqv = np.arange(1025, dtype=f32)
        lut = np.zeros((L + 1, 1025), f32)
        for t in range(L + 1):
            mu = f32(t) / f32(N)
            var = sqv / f32(N) - mu * mu
            lut[t] = np.asarray(jax.lax.rsqrt(jnp.asarray(var + f32(1e-5))))
    lut_flat = lut.reshape(-1, 1)

    def pack_kt(w):  # [HDIM, X] -> [128, JH*X]
        X = w.shape[1]
        return np.ascontiguousarray(
            w.reshape(JH, 128, X).transpose(1, 0, 2).reshape(128, JH * X))

    Whw = pack_kt(np.asarray(inputs["W_h2w"], f32))
    Whc = pack_kt(np.asarray(inputs["W_h2c"], f32)).astype(np.float16)
    Whgab = pack_kt(np.concatenate(
        [np.asarray(inputs["W_h2gates"], f32),
         np.asarray(inputs["W_h2ab"], f32)], axis=1))
    Whm = pack_kt(np.asarray(inputs["W_h2m"], f32)).astype(np.float16)
    Wrc = np.concatenate([
        np.asarray(inputs["W_r2c"], f32)[A:],
        np.asarray(inputs["W_r2gates"], f32)[A:],
        np.asarray(inputs["W_r2ab"], f32)[A:]], axis=1).astype(f32)  # [128,1029]
    Wrc16 = Wrc[:, :1024].astype(np.float16)
    Wm2A = np.ascontiguousarray(W_m2w[A:])                 # [128, 256]
    M0T = np.ascontiguousarray(
        M0.T.reshape(KT, 128, N).transpose(1, 0, 2).reshape(128, KT * N))
    base = np.asarray(inputs["atten_base"], f32)
    bmask = np.zeros((128, KT, BL, BL), f32)
    for kt in range(KT):
        for b in range(BL):
            bmask[:, kt, b, b] = base[kt * 128:(kt + 1) * 128]
    bmask = bmask.reshape(128, KT * BL * BL).astype(np.float16)
    Wicat = np.concatenate([
        np.asarray(inputs["W_i2w"], f32),
        np.asarray(inputs["W_i2c"], f32),
        np.asarray(inputs["W_i2gates"], f32),
        np.asarray(inputs["W_i2ab"], f32)], axis=1)  # [512, 1285]
    Wicat_p = np.ascontiguousarray(
        Wicat.reshape(JI, 128, 1285).transpose(1, 0, 2).reshape(128, JI * 1285))

    shared = {
        "Wicat": Wicat_p, "Whw": Whw, "Whc": Whc, "Whgab": Whgab,
        "Whm": Whm, "Wrc": Wrc, "Wm2A": Wm2A, "M0T": M0T, "bmask": bmask,
        "colsum16": np.tile(colsum[None, :], (BL, 1)),
        "iota16": np.tile(np.arange(N, dtype=f32)[None, :], (BL, 1)),
        "boff": (np.arange(BL, dtype=np.int32) * N)[:, None],
        "Tcat": T_cat, "Wu2w": np.asarray(inputs["W_u2w"], f32),
        "Wrc16": Wrc16,
        "slut": lut_flat,
        "sel5": _sel5(),
        "M0rows": np.ascontiguousarray(M0),
        "base16": np.ascontiguousarray(
            np.repeat(base.reshape(KT, 128).transpose(1, 0)[:, :, None], BL, 2
                      ).reshape(128, KT * BL)).astype(np.float16),
        "bsel128": np.tile(np.eye(BL, dtype=f32), (8, 1)),
    }

    inp = np.asarray(inputs["inp"], f32)
    hid = np.asarray(inputs["hid"], f32)
    in_maps = []
    for core in range(NCORES):
        b0 = core * BL
        bsl = slice(b0, b0 + BL)
        # inpT [128, (JI, L*BL)]: inpT[p, j, l*BL+b] = inp[l, b0+b, j*128+p]
        it = inp[:, bsl, :].transpose(2, 0, 1).reshape(JI, 128, L * BL)
        inpT = np.ascontiguousarray(it.transpose(1, 0, 2).reshape(128, JI * L * BL))
        # h0T [128, (JH, BL)]
        h0 = hid[bsl].T.reshape(JH, 128, BL).transpose(1, 0, 2).reshape(128, JH * BL)
        g_core = np.ascontiguousarray(
            g_read[:, bsl, :].reshape(L * BL, N))
        gdT = np.ascontiguousarray(
            gd[:, bsl, :].transpose(2, 0, 1).reshape(2, L * BL))
        m = dict(shared)
        m.update({
            "inpT": inpT, "h0T": np.ascontiguousarray(h0),
            "g_read": g_core, "gdT": gdT,
        })
        in_maps.append(m)
    return in_maps


def kernel(**inputs) -> np.ndarray:
    nc = _get_program()
    in_maps = _host_prep(inputs)
    res = run_bass_kernel_spmd(nc, in_maps, core_ids=list(range(NCORES)))
    outs = np.zeros((L, B, HDIM), np.float32)
    for core in range(NCORES):
        o = res.results[core]["outs"]  # [L, HDIM, BL]
        outs[:, core * BL:(core + 1) * BL, :] = o.transpose(0, 2, 1)
    return outs
